# revision 9
# baseline (speedup 1.0000x reference)
"""MoE (dense routing) Trainium2 kernel.

Math: out = softmax(x@Wg+bg) -weighted sum over experts of
      (gelu(x@W1[e]+b1[e]) @ W2[e] + b2[e]).

Strategy (data-parallel over 8 cores, 2048 tokens each):
  - Host pre-transposes x (xT [D, tokens]) and packs W1 as [D, E*H];
    all matmul operands are converted to bf16 on host (tolerance is
    2e-2 rel; bf16 lands ~4e-3). bf16 halves every DMA (the weight
    prologue is HBM-bandwidth-bound at ~350GB/s) and enables the PE's
    Fast Weight Load path (fp32 LDWEIGHTS at ~191ns was pacing L1's
    107ns matmuls).
  - Layer 1 runs "transposed": hT[ej, t] = sum_d W1p[d, ej] * xT[d, t]
    via matmuls with W1p chunks stationary and xT chunks moving ->
    hidden lands with ej on partitions, tokens on free dim.
  - b1 is applied as the ACT bias during the gelu (per-partition bias).
  - Gate: logitsT[e, t] accumulated the same way; exp fused with +bg on
    ACT; weights kept UNNORMALIZED (exp). The softmax denominator is
    applied at the very end as a per-token scale on the output copy
    (on DVE, so the ACT engine never loads the Copy table and PSUM
    output banks recycle without waiting on the gelu stream).
  - Scaled hidden shT[ej, t] = gelu_out * exp[e(ej), t] (DVE mul with a
    DMA partition-broadcast of the exp row).
  - Layer 2: out[t, o] = sum_ej shT[ej, t(chunk stationary)] @ W2p[ej, o]
    accumulated in PSUM over all ej chunks, seeded with expT @ b2
    (start=True) which realizes the sum_e w_e*b2[e] term. The two
    K=8 bias seeds of a 128-token slice run CONCURRENTLY in separate
    PE row groups (tile_position row tiling) -- exp and b2 are
    replicated at partition offset 32 so the second matmul's operands
    stream through array rows 32-63.
  - PE program order per iteration: l1(k) matmuls, gate logits(k+1),
    l2(k-1), softmax-denominator(k+1). The denominator matmul's
    stationary operand is ACT's exp output; putting it after l2 gives
    the ACT queue (16 gelus + an Exp<->Gelu table reload) a full l2
    stage of slack.
  - DMA prologue: first bytes on each queue are exactly what the PE
    consumes first (xts0+Wgs on SP; W1 chunks round-robined in
    consumption order across Pool/SP/ACT). Aggregate HBM is ~350GB/s,
    so the 4MB W1 takes ~12us; the PE starts on chunk 0 at ~9us and
    never outruns the stream.
No transposes on device at all.
"""

import numpy as np
from contextlib import ExitStack

import ml_dtypes
import orjson

import concourse.bass as bass
import concourse.bass2jax as bass2jax
import concourse.bass_utils as bass_utils
import concourse.tile as tile
from concourse import mybir
from concourse.bass_utils import run_bass_kernel_spmd

# The walrus build in this container rejects any instruction carrying more
# than one sync wait ("Too many sync wait commands", CoreV3GenImpl
# setupSyncWait), but the tile scheduler freely attaches several. Split the
# extras onto standalone single-wait EventSemaphore carriers placed just
# before the instruction (same engine, so program order is preserved).
_orig_compile_bir_kernel = bass_utils.compile_bir_kernel


def _split_multiwait_bir(bir_json):
    bir = orjson.loads(bir_json)
    changed = False
    for fn in bir.get("functions", []):
        for blk in fn.get("blocks", []):
            ins_list = blk.get("instructions")
            if not ins_list:
                continue
            out = []
            for inst in ins_list:
                si = inst.get("sync_info")
                if si:
                    waits = si.get("on_wait") or []
                    if len(waits) > 1:
                        changed = True
                        for k, w in enumerate(waits[:-1]):
                            carrier = {
                                "engine": inst["engine"],
                                "ins": [],
                                "outs": [],
                                "name": f"{inst['name']}_xw{k}",
                                "opcode": "EventSemaphore",
                                "sync_info": {"on_update": [], "on_wait": [w]},
                            }
                            if "debug" in inst:
                                carrier["debug"] = inst["debug"]
                            out.append(carrier)
                        si["on_wait"] = [waits[-1]]
                out.append(inst)
            blk["instructions"] = out
    return orjson.dumps(bir) if changed else bir_json


def _compile_bir_kernel_split(bir_json, tmpdir, neff_name="file.neff"):
    return _orig_compile_bir_kernel(_split_multiwait_bir(bir_json), tmpdir, neff_name)


bass_utils.compile_bir_kernel = _compile_bir_kernel_split
bass2jax.compile_bir_kernel = _compile_bir_kernel_split

N, D, H, O, E = 16384, 1024, 256, 1024, 8
NCORES = 8
NTOK = N // NCORES  # tokens per core
P = 128
T = 256  # token block size (moving free dim)
TS = T // P  # 128-token sub-blocks per block
NB = NTOK // T  # token blocks per core
DC = D // P  # d chunks (contraction, layer 1)
EJ = E * H  # packed hidden width
NEJ = EJ // P  # ej chunks (contraction, layer 2)
JC_PER_E = H // P  # ej chunks per expert
OH = O // 2  # layer-2 output half width (one PSUM bank)

FP = mybir.dt.float32
BF = mybir.dt.bfloat16
AF = mybir.ActivationFunctionType
NPBF = ml_dtypes.bfloat16


def _build_nc():
    nc = bass.Bass(enable_partition_id=False)
    xT = nc.dram_tensor("xT", [D, NTOK], BF, kind="ExternalInput")
    W1p = nc.dram_tensor("W1p", [D, EJ], BF, kind="ExternalInput")
    Wg = nc.dram_tensor("Wg", [D, E], BF, kind="ExternalInput")
    W2p = nc.dram_tensor("W2p", [EJ, O], BF, kind="ExternalInput")
    b1h = nc.dram_tensor("b1h", [P, NEJ], FP, kind="ExternalInput")
    bgh = nc.dram_tensor("bgh", [E, 1], FP, kind="ExternalInput")
    b2 = nc.dram_tensor("b2", [E, O], BF, kind="ExternalInput")
    out = nc.dram_tensor("out", [NTOK, O], FP, kind="ExternalOutput")

    with tile.TileContext(nc) as tc, ExitStack() as ctx:
        const = ctx.enter_context(tc.tile_pool(name="const", bufs=1))
        dpool = ctx.enter_context(tc.tile_pool(name="dram", bufs=2, space="DRAM"))
        xpool = ctx.enter_context(tc.tile_pool(name="xts", bufs=3))
        gpool = ctx.enter_context(tc.tile_pool(name="gelu", bufs=3))
        shpool = ctx.enter_context(tc.tile_pool(name="sh", bufs=2))
        bcpool = ctx.enter_context(tc.tile_pool(name="bc", bufs=2))
        epool = ctx.enter_context(tc.tile_pool(name="expp", bufs=3))
        opool = ctx.enter_context(tc.tile_pool(name="outp", bufs=3))
        rpool = ctx.enter_context(tc.tile_pool(name="rcp", bufs=3))
        ps_h = ctx.enter_context(tc.tile_pool(name="ps_h", bufs=3, space="PSUM"))
        ps_g = ctx.enter_context(tc.tile_pool(name="ps_g", bufs=1, space="PSUM"))
        ps_s = ctx.enter_context(tc.tile_pool(name="ps_s", bufs=1, space="PSUM"))
        ps_o = ctx.enter_context(tc.tile_pool(name="ps_o", bufs=3, space="PSUM"))

        W1s = const.tile([P, DC, EJ], BF)
        W1v = W1p.rearrange("(dc p) ej -> p dc ej", p=P)
        W2s = const.tile([P, NEJ, O], BF)
        W2v = W2p.rearrange("(ec p) o -> p ec o", p=P)
        Wgs = const.tile([P, DC, E], BF)
        b1s = const.tile([P, NEJ], FP)
        bgs = const.tile([E, 1], FP)
        # b2 rows replicated at partition offset 32 so the second bias-seed
        # matmul can run in PE row group 1 (its operands stream rows 32-63)
        b2s = const.tile([32 + E, O], BF)
        ones8 = const.tile([E, 1], BF)

        def load_xts(blk, split3=False):
            t0 = blk * T
            xts = xpool.tile([P, DC, T], BF, name=f"xts{blk}", tag="xts")
            xv = xT[:, t0 : t0 + T].rearrange("(dc p) t -> p dc t", p=P)
            if split3:
                nc.sync.dma_start(xts[:, 0:3, :], xv[:, 0:3, :])
                nc.scalar.dma_start(xts[:, 3:6, :], xv[:, 3:6, :])
                nc.gpsimd.dma_start(xts[:, 6:8, :], xv[:, 6:8, :])
            else:
                nc.sync.dma_start(xts[:, 0 : DC // 2, :], xv[:, 0 : DC // 2, :])
                nc.sync.dma_start(xts[:, DC // 2 :, :], xv[:, DC // 2 :, :])
            return xts

        # ---- prologue DMAs ----
        # HBM is ~350GB/s shared across the three queues, so every queue's
        # byte stream follows the PE's global consumption order: xts0 ->
        # Wgs/biases -> W1 chunks (each split across queues so chunk k fully
        # lands before chunk k+1 bytes move) -> xts1 -> W2. Pool carries the
        # block-0 exp broadcast between W1 c2 and c3 (it idles on ACT's exp
        # anyway), so W1 c0-c2 are 2-way splits on SP/ACT.
        xtss = {0: load_xts(0, split3=True)}
        nc.sync.dma_start(Wgs[:], Wg.rearrange("(dc p) e -> p dc e", p=P))
        nc.scalar.dma_start(b1s[:], b1h[:])
        nc.scalar.dma_start(bgs[:], bgh[:])
        nc.gpsimd.memset(ones8[:], 1.0)
        NCH = 8
        for c in range(3):
            sl = slice(c * (EJ // NCH), (c + 1) * (EJ // NCH))
            nc.sync.dma_start(W1s[:, 0:4, sl], W1v[:, 0:4, sl])
            nc.scalar.dma_start(W1s[:, 4:8, sl], W1v[:, 4:8, sl])

        def gate_logits(blk, xts):
            # gate logits (transposed): gt[e, t]
            gt = ps_g.tile([E, T], FP, name=f"gt{blk}", tag="gt")
            for dc in range(DC):
                nc.tensor.matmul(
                    gt[:],
                    Wgs[:, dc, :],
                    xts[:, dc, :],
                    start=(dc == 0),
                    stop=(dc == DC - 1),
                )
            return gt

        def gate_exp(blk, gt):
            # exp rows at partitions 0-7; rows 32-39 get a copy via the DRAM
            # bounce so the second bias-seed matmul can use row group 1
            expv = epool.tile([32 + E, T], BF, name=f"exp{blk}", tag="exp")
            nc.scalar.activation(expv[0:E, :], gt[:], AF.Exp, bias=bgs[:, 0:1])
            # broadcast exp rows across partitions for the hidden scaling
            # (partition-stride-0 DMA only legal from DRAM -> bounce there)
            expd = dpool.tile([E, T], BF, name=f"expd{blk}", tag="expd")
            nc.gpsimd.dma_start(expd[:], expv[0:E, :])
            nc.gpsimd.dma_start(expv[32 : 32 + E, :], expd[:])
            bc = bcpool.tile([P, E, T], BF, name=f"bc{blk}", tag="bc")
            for e in range(E):
                nc.gpsimd.dma_start(bc[:, e, :], expd[e : e + 1, :].to_broadcast((P, T)))
            return expv, bc

        def gate_denom(blk, expv):
            # softmax denominator, landed in token-partition layout via a
            # K=8 ones matmul; both 128-token halves into one PSUM tile
            s = ps_s.tile([P, TS], FP, name=f"s{blk}", tag="s")
            for ts in range(TS):
                nc.tensor.matmul(
                    s[:, ts : ts + 1],
                    expv[0:E, ts * P : (ts + 1) * P],
                    ones8[:],
                    start=True,
                    stop=True,
                )
            rcp = rpool.tile([P, TS], FP, name=f"rcp{blk}", tag="rcp")
            nc.vector.reciprocal(rcp[:], s[:])
            return rcp

        def l1_stage(blk, xts, bc):
            sh = shpool.tile([P, NEJ, T], BF, name=f"sh{blk}", tag="sh")
            for ejc in range(NEJ):
                ht = ps_h.tile([P, T], FP, name=f"ht{blk}_{ejc}", tag="ht")
                for dc in range(DC):
                    nc.tensor.matmul(
                        ht[:],
                        W1s[:, dc, ejc * P : (ejc + 1) * P],
                        xts[:, dc, :],
                        start=(dc == 0),
                        stop=(dc == DC - 1),
                    )
                g = gpool.tile([P, T], BF, name=f"g{blk}_{ejc}", tag="g")
                nc.scalar.activation(g[:], ht[:], AF.Gelu, bias=b1s[:, ejc : ejc + 1])
                nc.vector.tensor_tensor(
                    sh[:, ejc, :], g[:], bc[:, ejc // JC_PER_E, :], mybir.AluOpType.mult
                )
            return sh

        OUT_ENG = [nc.sync, nc.scalar, nc.gpsimd]

        def l2_stage(blk, sh, expv, rcp, last=False):
            t0 = blk * T
            for ts in range(TS):
                tsl = slice(ts * P, (ts + 1) * P)
                # the two K=8 bias seeds run concurrently in row groups 0/1
                ops = []
                for half in range(2):
                    o0 = half * OH
                    op = ps_o.tile(
                        [P, OH], FP, name=f"ops{blk}_{ts}_{half}", tag="ops"
                    )
                    r = 32 * half
                    nc.tensor.matmul(
                        op[:],
                        expv[r : r + E, tsl],
                        b2s[r : r + E, o0 : o0 + OH],
                        start=True,
                        stop=False,
                        tile_position=(r, 0),
                    )
                    ops.append(op)
                for half in range(2):
                    o0 = half * OH
                    for ejc in range(NEJ):
                        nc.tensor.matmul(
                            ops[half][:],
                            sh[:, ejc, tsl],
                            W2s[:, ejc, o0 : o0 + OH],
                            start=False,
                            stop=(ejc == NEJ - 1),
                        )
                for half in range(2):
                    o0 = half * OH
                    outsb = opool.tile(
                        [P, OH], FP, name=f"o{blk}_{ts}_{half}", tag="o"
                    )
                    # per-token 1/sum_e exp scale; DVE so ACT stays on gelu
                    nc.vector.tensor_scalar_mul(
                        outsb[:], ops[half][:], rcp[:, ts : ts + 1]
                    )
                    eng = OUT_ENG[(2 * ts + half) % 3] if last else OUT_ENG[(ts + half) % 2]
                    eng.dma_start(
                        out[t0 + ts * P : t0 + (ts + 1) * P, o0 : o0 + OH], outsb[:]
                    )

        with tc.high_priority():
            gt0 = gate_logits(0, xtss[0])
            expv0, bc0 = gate_exp(0, gt0)
            rcp0 = gate_denom(0, expv0)
        states = {0: (expv0, rcp0, bc0)}
        for c in range(3, NCH):
            sl = slice(c * (EJ // NCH), (c + 1) * (EJ // NCH))
            nc.sync.dma_start(W1s[:, 0:3, sl], W1v[:, 0:3, sl])
            nc.scalar.dma_start(W1s[:, 3:6, sl], W1v[:, 3:6, sl])
            nc.gpsimd.dma_start(W1s[:, 6:8, sl], W1v[:, 6:8, sl])
        nc.scalar.dma_start(b2s[0:E, :], b2[:])
        nc.scalar.dma_start(b2s[32 : 32 + E, :], b2[:])
        xtss[1] = load_xts(1, split3=True)
        W2_ENG = [nc.sync, nc.scalar, nc.gpsimd]
        for sg in range(NEJ // 2):
            sl = slice(sg * 2, sg * 2 + 2)
            W2_ENG[sg % 3].dma_start(W2s[:, sl, :], W2v[:, sl, :])

        sh_prev = None
        for blk in range(NB):
            sh = l1_stage(blk, xtss[blk], states[blk][2])
            if blk + 2 < NB:
                xtss[blk + 2] = load_xts(blk + 2)
            gt = gate_logits(blk + 1, xtss[blk + 1]) if blk + 1 < NB else None
            if blk >= 1:
                l2_stage(blk - 1, sh_prev, states[blk - 1][0], states[blk - 1][1])
            if blk + 1 < NB:
                expv, bc = gate_exp(blk + 1, gt)
                rcp = gate_denom(blk + 1, expv)
                states[blk + 1] = (expv, rcp, bc)
            sh_prev = sh
        l2_stage(NB - 1, sh_prev, states[NB - 1][0], states[NB - 1][1], last=True)
    return nc


_CACHE = {}


def kernel(**inputs):
    x = np.asarray(inputs["x"], dtype=np.float32)
    W1 = np.asarray(inputs["W1"], dtype=np.float32)
    b1 = np.asarray(inputs["b1"], dtype=np.float32)
    W2 = np.asarray(inputs["W2"], dtype=np.float32)
    b2 = np.asarray(inputs["b2"], dtype=np.float32)
    Wg = np.asarray(inputs["Wg"], dtype=np.float32)
    bg = np.asarray(inputs["bg"], dtype=np.float32)

    W1p = np.ascontiguousarray(W1.transpose(1, 0, 2).reshape(D, EJ)).astype(NPBF)
    W2p = np.ascontiguousarray(W2.reshape(EJ, O)).astype(NPBF)
    b1h = np.ascontiguousarray(b1.reshape(EJ).reshape(NEJ, P).T)
    bgh = np.ascontiguousarray(bg.reshape(E, 1))
    Wg16 = Wg.astype(NPBF)
    b216 = b2.astype(NPBF)

    if "nc" not in _CACHE:
        _CACHE["nc"] = _build_nc()
    nc = _CACHE["nc"]

    in_maps = []
    for c in range(NCORES):
        xs = x[c * NTOK : (c + 1) * NTOK]
        in_maps.append(
            {
                "xT": np.ascontiguousarray(xs.T).astype(NPBF),
                "W1p": W1p,
                "Wg": Wg16,
                "W2p": W2p,
                "b1h": b1h,
                "bgh": bgh,
                "b2": b216,
            }
        )

    res = run_bass_kernel_spmd(nc, in_maps, list(range(NCORES)))
    kernel.last = res
    return np.concatenate([res.results[c]["out"] for c in range(NCORES)], axis=0)


# revision 14
# speedup vs baseline: 1.0269x; 1.0269x over previous
"""MoE (dense routing) Trainium2 kernel.

Math: out = softmax(x@Wg+bg) -weighted sum over experts of
      (gelu(x@W1[e]+b1[e]) @ W2[e] + b2[e]).

Strategy (data-parallel over 8 cores, 2048 tokens each):
  - Host pre-transposes x (xT [D, tokens]) and packs W1 as [D, E*H];
    all matmul operands are converted to bf16 on host (tolerance is
    2e-2 rel; bf16 lands ~4e-3). bf16 halves every DMA (the weight
    prologue is HBM-bandwidth-bound at ~350GB/s) and enables the PE's
    Fast Weight Load path (fp32 LDWEIGHTS at ~191ns was pacing L1's
    107ns matmuls).
  - Layer 1 runs "transposed": hT[ej, t] = sum_d W1p[d, ej] * xT[d, t]
    via matmuls with W1p chunks stationary and xT chunks moving ->
    hidden lands with ej on partitions, tokens on free dim.
  - b1 is applied as the ACT bias during the gelu (per-partition bias).
  - Gate: logitsT[e, t] accumulated the same way; exp fused with +bg on
    ACT; weights kept UNNORMALIZED (exp). The softmax denominator is
    applied at the very end as a per-token scale on the output copy
    (on DVE, so the ACT engine never loads the Copy table and PSUM
    output banks recycle without waiting on the gelu stream).
  - Scaled hidden shT[ej, t] = gelu_out * exp[e(ej), t] (DVE mul with a
    DMA partition-broadcast of the exp row).
  - Layer 2: out[t, o] = sum_ej shT[ej, t(chunk stationary)] @ W2p[ej, o]
    accumulated in PSUM over all ej chunks, seeded with expT @ b2
    (start=True) which realizes the sum_e w_e*b2[e] term. The two
    K=8 bias seeds of a 128-token slice run CONCURRENTLY in separate
    PE row groups (tile_position row tiling) -- exp and b2 are
    replicated at partition offset 32 so the second matmul's operands
    stream through array rows 32-63.
  - PE program order per iteration: l1(k) matmuls, gate logits(k+1),
    l2(k-1), softmax-denominator(k+1). The denominator matmul's
    stationary operand is ACT's exp output; putting it after l2 gives
    the ACT queue (16 gelus + an Exp<->Gelu table reload) a full l2
    stage of slack.
  - DMA prologue: first bytes on each queue are exactly what the PE
    consumes first (xts0+Wgs on SP; W1 chunks round-robined in
    consumption order across Pool/SP/ACT). Aggregate HBM is ~350GB/s,
    so the 4MB W1 takes ~12us; the PE starts on chunk 0 at ~9us and
    never outruns the stream.
No transposes on device at all.
"""

import numpy as np
from contextlib import ExitStack

import ml_dtypes
import orjson

import concourse.bass as bass
import concourse.bass2jax as bass2jax
import concourse.bass_utils as bass_utils
import concourse.tile as tile
from concourse import mybir
from concourse.bass_utils import run_bass_kernel_spmd

# The walrus build in this container rejects any instruction carrying more
# than one sync wait ("Too many sync wait commands", CoreV3GenImpl
# setupSyncWait), but the tile scheduler freely attaches several. Split the
# extras onto standalone single-wait EventSemaphore carriers placed just
# before the instruction (same engine, so program order is preserved).
_orig_compile_bir_kernel = bass_utils.compile_bir_kernel


def _split_multiwait_bir(bir_json):
    bir = orjson.loads(bir_json)
    changed = False
    for fn in bir.get("functions", []):
        for blk in fn.get("blocks", []):
            ins_list = blk.get("instructions")
            if not ins_list:
                continue
            out = []
            for inst in ins_list:
                si = inst.get("sync_info")
                if si:
                    waits = si.get("on_wait") or []
                    if len(waits) > 1:
                        changed = True
                        for k, w in enumerate(waits[:-1]):
                            carrier = {
                                "engine": inst["engine"],
                                "ins": [],
                                "outs": [],
                                "name": f"{inst['name']}_xw{k}",
                                "opcode": "EventSemaphore",
                                "sync_info": {"on_update": [], "on_wait": [w]},
                            }
                            if "debug" in inst:
                                carrier["debug"] = inst["debug"]
                            out.append(carrier)
                        si["on_wait"] = [waits[-1]]
                out.append(inst)
            blk["instructions"] = out
    return orjson.dumps(bir) if changed else bir_json


def _compile_bir_kernel_split(bir_json, tmpdir, neff_name="file.neff"):
    return _orig_compile_bir_kernel(_split_multiwait_bir(bir_json), tmpdir, neff_name)


bass_utils.compile_bir_kernel = _compile_bir_kernel_split
bass2jax.compile_bir_kernel = _compile_bir_kernel_split

N, D, H, O, E = 16384, 1024, 256, 1024, 8
NCORES = 8
NTOK = N // NCORES  # tokens per core
P = 128
T = 256  # token block size (moving free dim)
TS = T // P  # 128-token sub-blocks per block
NB = NTOK // T  # token blocks per core
DC = D // P  # d chunks (contraction, layer 1)
EJ = E * H  # packed hidden width
NEJ = EJ // P  # ej chunks (contraction, layer 2)
JC_PER_E = H // P  # ej chunks per expert
OH = O // 2  # layer-2 output half width (one PSUM bank)

FP = mybir.dt.float32
BF = mybir.dt.bfloat16
AF = mybir.ActivationFunctionType
NPBF = ml_dtypes.bfloat16


def _build_nc():
    nc = bass.Bass(enable_partition_id=False)
    xT = nc.dram_tensor("xT", [D, NTOK], BF, kind="ExternalInput")
    W1p = nc.dram_tensor("W1p", [D, EJ], BF, kind="ExternalInput")
    Wg = nc.dram_tensor("Wg", [D, E], BF, kind="ExternalInput")
    W2p = nc.dram_tensor("W2p", [EJ, O], BF, kind="ExternalInput")
    b1h = nc.dram_tensor("b1h", [P, NEJ], FP, kind="ExternalInput")
    bgh = nc.dram_tensor("bgh", [E, 1], FP, kind="ExternalInput")
    b2 = nc.dram_tensor("b2", [E, O], BF, kind="ExternalInput")
    out = nc.dram_tensor("out", [NTOK, O], FP, kind="ExternalOutput")

    with tile.TileContext(nc) as tc, ExitStack() as ctx:
        const = ctx.enter_context(tc.tile_pool(name="const", bufs=1))
        dpool = ctx.enter_context(tc.tile_pool(name="dram", bufs=2, space="DRAM"))
        xpool = ctx.enter_context(tc.tile_pool(name="xts", bufs=3))
        gpool = ctx.enter_context(tc.tile_pool(name="gelu", bufs=3))
        shpool = ctx.enter_context(tc.tile_pool(name="sh", bufs=2))
        bcpool = ctx.enter_context(tc.tile_pool(name="bc", bufs=2))
        epool = ctx.enter_context(tc.tile_pool(name="expp", bufs=3))
        opool = ctx.enter_context(tc.tile_pool(name="outp", bufs=3))
        rpool = ctx.enter_context(tc.tile_pool(name="rcp", bufs=3))
        ps_h = ctx.enter_context(tc.tile_pool(name="ps_h", bufs=3, space="PSUM"))
        ps_g = ctx.enter_context(tc.tile_pool(name="ps_g", bufs=1, space="PSUM"))
        ps_s = ctx.enter_context(tc.tile_pool(name="ps_s", bufs=1, space="PSUM"))
        ps_o = ctx.enter_context(tc.tile_pool(name="ps_o", bufs=3, space="PSUM"))

        W1s = const.tile([P, DC, EJ], BF)
        W1v = W1p.rearrange("(dc p) ej -> p dc ej", p=P)
        W2s = const.tile([P, NEJ, O], BF)
        W2v = W2p.rearrange("(ec p) o -> p ec o", p=P)
        Wgs = const.tile([P, DC, E], BF)
        b1s = const.tile([P, NEJ], FP)
        bgs = const.tile([E, 1], FP)
        # b2 rows replicated at partition offset 32 so the second bias-seed
        # matmul can run in PE row group 1 (its operands stream rows 32-63)
        b2s = const.tile([32 + E, O], BF)
        ones8 = const.tile([E, 1], BF)

        def load_xts(blk, split3=False):
            t0 = blk * T
            xts = xpool.tile([P, DC, T], BF, name=f"xts{blk}", tag="xts")
            xv = xT[:, t0 : t0 + T].rearrange("(dc p) t -> p dc t", p=P)
            if split3:
                nc.sync.dma_start(xts[:, 0:3, :], xv[:, 0:3, :])
                nc.scalar.dma_start(xts[:, 3:6, :], xv[:, 3:6, :])
                nc.gpsimd.dma_start(xts[:, 6:8, :], xv[:, 6:8, :])
            else:
                nc.sync.dma_start(xts[:, 0 : DC // 2, :], xv[:, 0 : DC // 2, :])
                nc.sync.dma_start(xts[:, DC // 2 :, :], xv[:, DC // 2 :, :])
            return xts

        # ---- prologue DMAs ----
        # HBM is ~330GB/s shared across the three queues, so every queue's
        # byte stream follows the PE's global consumption order: xts0 ->
        # Wgs/biases -> W1 chunks (each split across queues so chunk k fully
        # lands before chunk k+1 bytes move) -> xts1 -> W2. Pool carries the
        # block-0 exp broadcast between W1 c2 and c3 (it idles on ACT's exp
        # anyway).
        xtss = {0: load_xts(0, split3=True)}
        nc.sync.dma_start(Wgs[:], Wg.rearrange("(dc p) e -> p dc e", p=P))
        nc.scalar.dma_start(b1s[:], b1h[:])
        nc.scalar.dma_start(bgs[:], bgh[:])
        nc.gpsimd.memset(ones8[:], 1.0)
        NCH = 8
        for c in range(3):
            sl = slice(c * (EJ // NCH), (c + 1) * (EJ // NCH))
            nc.sync.dma_start(W1s[:, 0:3, sl], W1v[:, 0:3, sl])
            nc.scalar.dma_start(W1s[:, 3:6, sl], W1v[:, 3:6, sl])
            nc.gpsimd.dma_start(W1s[:, 6:8, sl], W1v[:, 6:8, sl])

        def gate_logits(blk, xts):
            # gate logits (transposed): gt[e, t]
            gt = ps_g.tile([E, T], FP, name=f"gt{blk}", tag="gt")
            for dc in range(DC):
                nc.tensor.matmul(
                    gt[:],
                    Wgs[:, dc, :],
                    xts[:, dc, :],
                    start=(dc == 0),
                    stop=(dc == DC - 1),
                )
            return gt

        def gate_exp(blk, gt):
            # exp rows at partitions 0-7; rows 32-39 get a copy via the DRAM
            # bounce so the second bias-seed matmul can use row group 1
            expv = epool.tile([32 + E, T], BF, name=f"exp{blk}", tag="exp")
            nc.scalar.activation(expv[0:E, :], gt[:], AF.Exp, bias=bgs[:, 0:1])
            # broadcast exp rows across partitions for the hidden scaling
            # (partition-stride-0 DMA only legal from DRAM -> bounce there)
            expd = dpool.tile([E, T], BF, name=f"expd{blk}", tag="expd")
            nc.gpsimd.dma_start(expd[:], expv[0:E, :])
            nc.gpsimd.dma_start(expv[32 : 32 + E, :], expd[:])
            bc = bcpool.tile([P, E, T], BF, name=f"bc{blk}", tag="bc")
            for e in range(E):
                nc.gpsimd.dma_start(bc[:, e, :], expd[e : e + 1, :].to_broadcast((P, T)))
            return expv, bc

        def gate_denom(blk, expv):
            # softmax denominator, landed in token-partition layout via a
            # K=8 ones matmul; both 128-token halves into one PSUM tile
            s = ps_s.tile([P, TS], FP, name=f"s{blk}", tag="s")
            for ts in range(TS):
                nc.tensor.matmul(
                    s[:, ts : ts + 1],
                    expv[0:E, ts * P : (ts + 1) * P],
                    ones8[:],
                    start=True,
                    stop=True,
                )
            rcp = rpool.tile([P, TS], FP, name=f"rcp{blk}", tag="rcp")
            nc.vector.reciprocal(rcp[:], s[:])
            return rcp

        def l1_stage(blk, xts, bc):
            sh = shpool.tile([P, NEJ, T], BF, name=f"sh{blk}", tag="sh")
            for ejc in range(NEJ):
                ht = ps_h.tile([P, T], FP, name=f"ht{blk}_{ejc}", tag="ht")
                for dc in range(DC):
                    nc.tensor.matmul(
                        ht[:],
                        W1s[:, dc, ejc * P : (ejc + 1) * P],
                        xts[:, dc, :],
                        start=(dc == 0),
                        stop=(dc == DC - 1),
                    )
                g = gpool.tile([P, T], BF, name=f"g{blk}_{ejc}", tag="g")
                nc.scalar.activation(g[:], ht[:], AF.Gelu, bias=b1s[:, ejc : ejc + 1])
                nc.vector.tensor_tensor(
                    sh[:, ejc, :], g[:], bc[:, ejc // JC_PER_E, :], mybir.AluOpType.mult
                )
            return sh

        # outputs never ride Pool: the software-DGE teardown DRAIN (~8.7us)
        # runs after Pool's last DMA, so a late Pool DMA lands it in the
        # measured tail
        OUT_ENG = [nc.sync, nc.scalar]

        def l2_stage(blk, sh, expv, rcp, last=False):
            t0 = blk * T
            for ts in range(TS):
                tsl = slice(ts * P, (ts + 1) * P)
                # the two K=8 bias seeds run concurrently in row groups 0/1
                ops = []
                for half in range(2):
                    o0 = half * OH
                    op = ps_o.tile(
                        [P, OH], FP, name=f"ops{blk}_{ts}_{half}", tag="ops"
                    )
                    r = 32 * half
                    nc.tensor.matmul(
                        op[:],
                        expv[r : r + E, tsl],
                        b2s[r : r + E, o0 : o0 + OH],
                        start=True,
                        stop=False,
                        tile_position=(r, 0),
                    )
                    ops.append(op)
                for half in range(2):
                    o0 = half * OH
                    for ejc in range(NEJ):
                        nc.tensor.matmul(
                            ops[half][:],
                            sh[:, ejc, tsl],
                            W2s[:, ejc, o0 : o0 + OH],
                            start=False,
                            stop=(ejc == NEJ - 1),
                        )
                for half in range(2):
                    o0 = half * OH
                    outsb = opool.tile(
                        [P, OH], FP, name=f"o{blk}_{ts}_{half}", tag="o"
                    )
                    # per-token 1/sum_e exp scale; DVE so ACT stays on gelu
                    nc.vector.tensor_scalar_mul(
                        outsb[:], ops[half][:], rcp[:, ts : ts + 1]
                    )
                    OUT_ENG[(ts + half) % 2].dma_start(
                        out[t0 + ts * P : t0 + (ts + 1) * P, o0 : o0 + OH], outsb[:]
                    )

        with tc.high_priority():
            gt0 = gate_logits(0, xtss[0])
            expv0, bc0 = gate_exp(0, gt0)
            # HAM warm-up bridge: the PE sits out ~2us here waiting for W1
            # chunk 0 (HBM-bound), which would keep the clock gate at
            # 1.2GHz deep into L1. Burn the wait on throwaway matmuls that
            # depend only on already-resident data so the activity monitor
            # sees a busy PE and releases full clock by the time real L1
            # work starts.
            dummy = ps_g.tile([E, T], FP, name="dummy", tag="gt")
            for k in range(10):
                nc.tensor.matmul(
                    dummy[:],
                    Wgs[:, k % DC, :],
                    xtss[0][:, (k + 1) % DC, :],
                    start=True,
                    stop=True,
                )
            rcp0 = gate_denom(0, expv0)
        states = {0: (expv0, rcp0, bc0)}
        for c in range(3, NCH):
            sl = slice(c * (EJ // NCH), (c + 1) * (EJ // NCH))
            nc.sync.dma_start(W1s[:, 0:3, sl], W1v[:, 0:3, sl])
            nc.scalar.dma_start(W1s[:, 3:6, sl], W1v[:, 3:6, sl])
            nc.gpsimd.dma_start(W1s[:, 6:8, sl], W1v[:, 6:8, sl])
        nc.scalar.dma_start(b2s[0:E, :], b2[:])
        nc.scalar.dma_start(b2s[32 : 32 + E, :], b2[:])
        xtss[1] = load_xts(1, split3=True)
        W2_ENG = [nc.sync, nc.scalar, nc.gpsimd]
        for sg in range(NEJ // 2):
            sl = slice(sg * 2, sg * 2 + 2)
            W2_ENG[sg % 3].dma_start(W2s[:, sl, :], W2v[:, sl, :])

        sh_prev = None
        for blk in range(NB):
            sh = l1_stage(blk, xtss[blk], states[blk][2])
            if blk + 2 < NB:
                xtss[blk + 2] = load_xts(blk + 2)
            gt = gate_logits(blk + 1, xtss[blk + 1]) if blk + 1 < NB else None
            if blk >= 1:
                l2_stage(blk - 1, sh_prev, states[blk - 1][0], states[blk - 1][1])
            if blk + 1 < NB:
                expv, bc = gate_exp(blk + 1, gt)
                rcp = gate_denom(blk + 1, expv)
                states[blk + 1] = (expv, rcp, bc)
            sh_prev = sh
        l2_stage(NB - 1, sh_prev, states[NB - 1][0], states[NB - 1][1])
    return nc


_CACHE = {}


def kernel(**inputs):
    x = np.asarray(inputs["x"], dtype=np.float32)
    W1 = np.asarray(inputs["W1"], dtype=np.float32)
    b1 = np.asarray(inputs["b1"], dtype=np.float32)
    W2 = np.asarray(inputs["W2"], dtype=np.float32)
    b2 = np.asarray(inputs["b2"], dtype=np.float32)
    Wg = np.asarray(inputs["Wg"], dtype=np.float32)
    bg = np.asarray(inputs["bg"], dtype=np.float32)

    W1p = np.ascontiguousarray(W1.transpose(1, 0, 2).reshape(D, EJ)).astype(NPBF)
    W2p = np.ascontiguousarray(W2.reshape(EJ, O)).astype(NPBF)
    b1h = np.ascontiguousarray(b1.reshape(EJ).reshape(NEJ, P).T)
    bgh = np.ascontiguousarray(bg.reshape(E, 1))
    Wg16 = Wg.astype(NPBF)
    b216 = b2.astype(NPBF)

    if "nc" not in _CACHE:
        _CACHE["nc"] = _build_nc()
    nc = _CACHE["nc"]

    in_maps = []
    for c in range(NCORES):
        xs = x[c * NTOK : (c + 1) * NTOK]
        in_maps.append(
            {
                "xT": np.ascontiguousarray(xs.T).astype(NPBF),
                "W1p": W1p,
                "Wg": Wg16,
                "W2p": W2p,
                "b1h": b1h,
                "bgh": bgh,
                "b2": b216,
            }
        )

    res = run_bass_kernel_spmd(nc, in_maps, list(range(NCORES)))
    kernel.last = res
    return np.concatenate([res.results[c]["out"] for c in range(NCORES)], axis=0)


# revision 17
# speedup vs baseline: 1.0334x; 1.0064x over previous
"""MoE (dense routing) Trainium2 kernel.

Math: out = softmax(x@Wg+bg) -weighted sum over experts of
      (gelu(x@W1[e]+b1[e]) @ W2[e] + b2[e]).

Strategy (data-parallel over 8 cores, 2048 tokens each):
  - Host pre-transposes x (xT [D, tokens]) and packs W1 as [D, E*H];
    all matmul operands are converted to bf16 on host (tolerance is
    2e-2 rel; bf16 lands ~4e-3). bf16 halves every DMA (the weight
    prologue is HBM-bandwidth-bound at ~350GB/s) and enables the PE's
    Fast Weight Load path (fp32 LDWEIGHTS at ~191ns was pacing L1's
    107ns matmuls).
  - Layer 1 runs "transposed": hT[ej, t] = sum_d W1p[d, ej] * xT[d, t]
    via matmuls with W1p chunks stationary and xT chunks moving ->
    hidden lands with ej on partitions, tokens on free dim.
  - b1 is applied as the ACT bias during the gelu (per-partition bias).
  - Gate: logitsT[e, t] accumulated the same way; exp fused with +bg on
    ACT; weights kept UNNORMALIZED (exp). The softmax denominator is
    applied at the very end as a per-token scale on the output copy
    (on DVE, so the ACT engine never loads the Copy table and PSUM
    output banks recycle without waiting on the gelu stream).
  - Scaled hidden shT[ej, t] = gelu_out * exp[e(ej), t] (DVE mul with a
    DMA partition-broadcast of the exp row).
  - Layer 2: out[t, o] = sum_ej shT[ej, t(chunk stationary)] @ W2p[ej, o]
    accumulated in PSUM over all ej chunks, seeded with expT @ b2
    (start=True) which realizes the sum_e w_e*b2[e] term. The two
    K=8 bias seeds of a 128-token slice run CONCURRENTLY in separate
    PE row groups (tile_position row tiling) -- exp and b2 are
    replicated at partition offset 32 so the second matmul's operands
    stream through array rows 32-63.
  - PE program order per iteration: l1(k) matmuls, gate logits(k+1),
    l2(k-1), softmax-denominator(k+1). The denominator matmul's
    stationary operand is ACT's exp output; putting it after l2 gives
    the ACT queue (16 gelus + an Exp<->Gelu table reload) a full l2
    stage of slack.
  - DMA prologue: first bytes on each queue are exactly what the PE
    consumes first (xts0+Wgs on SP; W1 chunks round-robined in
    consumption order across Pool/SP/ACT). Aggregate HBM is ~350GB/s,
    so the 4MB W1 takes ~12us; the PE starts on chunk 0 at ~9us and
    never outruns the stream.
No transposes on device at all.
"""

import numpy as np
from contextlib import ExitStack

import ml_dtypes
import orjson

import concourse.bass as bass
import concourse.bass2jax as bass2jax
import concourse.bass_utils as bass_utils
import concourse.tile as tile
from concourse import mybir
from concourse.bass_utils import run_bass_kernel_spmd

# The walrus build in this container rejects any instruction carrying more
# than one sync wait ("Too many sync wait commands", CoreV3GenImpl
# setupSyncWait), but the tile scheduler freely attaches several. Split the
# extras onto standalone single-wait EventSemaphore carriers placed just
# before the instruction (same engine, so program order is preserved).
_orig_compile_bir_kernel = bass_utils.compile_bir_kernel


def _split_multiwait_bir(bir_json):
    bir = orjson.loads(bir_json)
    changed = False
    for fn in bir.get("functions", []):
        for blk in fn.get("blocks", []):
            ins_list = blk.get("instructions")
            if not ins_list:
                continue
            out = []
            for inst in ins_list:
                si = inst.get("sync_info")
                if si:
                    waits = si.get("on_wait") or []
                    if len(waits) > 1:
                        changed = True
                        for k, w in enumerate(waits[:-1]):
                            carrier = {
                                "engine": inst["engine"],
                                "ins": [],
                                "outs": [],
                                "name": f"{inst['name']}_xw{k}",
                                "opcode": "EventSemaphore",
                                "sync_info": {"on_update": [], "on_wait": [w]},
                            }
                            if "debug" in inst:
                                carrier["debug"] = inst["debug"]
                            out.append(carrier)
                        si["on_wait"] = [waits[-1]]
                out.append(inst)
            blk["instructions"] = out
    return orjson.dumps(bir) if changed else bir_json


def _compile_bir_kernel_split(bir_json, tmpdir, neff_name="file.neff"):
    return _orig_compile_bir_kernel(_split_multiwait_bir(bir_json), tmpdir, neff_name)


bass_utils.compile_bir_kernel = _compile_bir_kernel_split
bass2jax.compile_bir_kernel = _compile_bir_kernel_split

N, D, H, O, E = 16384, 1024, 256, 1024, 8
NCORES = 8
NTOK = N // NCORES  # tokens per core
P = 128
T = 256  # token block size (moving free dim)
TS = T // P  # 128-token sub-blocks per block
NB = NTOK // T  # token blocks per core
DC = D // P  # d chunks (contraction, layer 1)
EJ = E * H  # packed hidden width
NEJ = EJ // P  # ej chunks (contraction, layer 2)
JC_PER_E = H // P  # ej chunks per expert
OH = O // 2  # layer-2 output half width (one PSUM bank)

FP = mybir.dt.float32
BF = mybir.dt.bfloat16
AF = mybir.ActivationFunctionType
NPBF = ml_dtypes.bfloat16


def _build_nc():
    nc = bass.Bass(enable_partition_id=False)
    xT = nc.dram_tensor("xT", [D, NTOK], BF, kind="ExternalInput")
    W1p = nc.dram_tensor("W1p", [D, EJ], BF, kind="ExternalInput")
    Wg = nc.dram_tensor("Wg", [D, E], BF, kind="ExternalInput")
    W2p = nc.dram_tensor("W2p", [EJ, O], BF, kind="ExternalInput")
    b1h = nc.dram_tensor("b1h", [P, NEJ], FP, kind="ExternalInput")
    bgh = nc.dram_tensor("bgh", [E, 1], FP, kind="ExternalInput")
    b2 = nc.dram_tensor("b2", [E, O], BF, kind="ExternalInput")
    out = nc.dram_tensor("out", [NTOK, O], FP, kind="ExternalOutput")

    with tile.TileContext(nc) as tc, ExitStack() as ctx:
        const = ctx.enter_context(tc.tile_pool(name="const", bufs=1))
        dpool = ctx.enter_context(tc.tile_pool(name="dram", bufs=2, space="DRAM"))
        xpool = ctx.enter_context(tc.tile_pool(name="xts", bufs=3))
        gpool = ctx.enter_context(tc.tile_pool(name="gelu", bufs=3))
        shpool = ctx.enter_context(tc.tile_pool(name="sh", bufs=2))
        bcpool = ctx.enter_context(tc.tile_pool(name="bc", bufs=2))
        epool = ctx.enter_context(tc.tile_pool(name="expp", bufs=3))
        opool = ctx.enter_context(tc.tile_pool(name="outp", bufs=3))
        rpool = ctx.enter_context(tc.tile_pool(name="rcp", bufs=3))
        ps_h = ctx.enter_context(tc.tile_pool(name="ps_h", bufs=3, space="PSUM"))
        ps_g = ctx.enter_context(tc.tile_pool(name="ps_g", bufs=1, space="PSUM"))
        ps_s = ctx.enter_context(tc.tile_pool(name="ps_s", bufs=1, space="PSUM"))
        ps_o = ctx.enter_context(tc.tile_pool(name="ps_o", bufs=3, space="PSUM"))

        W1s = const.tile([P, DC, EJ], BF)
        W1v = W1p.rearrange("(dc p) ej -> p dc ej", p=P)
        W2s = const.tile([P, NEJ, O], BF)
        W2v = W2p.rearrange("(ec p) o -> p ec o", p=P)
        Wgs = const.tile([P, DC, E], BF)
        b1s = const.tile([P, NEJ], FP)
        bgs = const.tile([E, 1], FP)
        # b2 rows replicated at partition offset 32 so the second bias-seed
        # matmul can run in PE row group 1 (its operands stream rows 32-63)
        b2s = const.tile([32 + E, O], BF)
        ones8 = const.tile([E, 1], BF)

        def load_xts(blk, split3=False):
            t0 = blk * T
            xts = xpool.tile([P, DC, T], BF, name=f"xts{blk}", tag="xts")
            xv = xT[:, t0 : t0 + T].rearrange("(dc p) t -> p dc t", p=P)
            if split3:
                nc.sync.dma_start(xts[:, 0:3, :], xv[:, 0:3, :])
                nc.scalar.dma_start(xts[:, 3:6, :], xv[:, 3:6, :])
                nc.gpsimd.dma_start(xts[:, 6:8, :], xv[:, 6:8, :])
            else:
                nc.sync.dma_start(xts[:, 0 : DC // 2, :], xv[:, 0 : DC // 2, :])
                nc.sync.dma_start(xts[:, DC // 2 :, :], xv[:, DC // 2 :, :])
            return xts

        # ---- prologue DMAs ----
        # HBM is ~330GB/s shared across the three queues, so every queue's
        # byte stream follows the PE's global consumption order: xts0 ->
        # Wgs/biases -> W1 chunks (each split across queues so chunk k fully
        # lands before chunk k+1 bytes move) -> xts1 -> W2. Pool carries the
        # block-0 exp broadcast between W1 c2 and c3 (it idles on ACT's exp
        # anyway).
        xtss = {0: load_xts(0, split3=True)}
        nc.sync.dma_start(Wgs[:], Wg.rearrange("(dc p) e -> p dc e", p=P))
        nc.scalar.dma_start(b1s[:], b1h[:])
        nc.scalar.dma_start(bgs[:], bgh[:])
        nc.gpsimd.memset(ones8[:], 1.0)
        # SP's DGE is dispatched by the otherwise-idle SP engine, so it gets
        # the biggest share; ACT's DGE competes with the gelu stream for the
        # ACT sequencer, so it only carries early bytes.
        NCH = 8
        for c in range(3):
            sl = slice(c * (EJ // NCH), (c + 1) * (EJ // NCH))
            nc.sync.dma_start(W1s[:, 0:4, sl], W1v[:, 0:4, sl])
            nc.scalar.dma_start(W1s[:, 4:6, sl], W1v[:, 4:6, sl])
            nc.gpsimd.dma_start(W1s[:, 6:8, sl], W1v[:, 6:8, sl])

        def gate_logits(blk, xts):
            # gate logits (transposed): gt[e, t]
            gt = ps_g.tile([E, T], FP, name=f"gt{blk}", tag="gt")
            for dc in range(DC):
                nc.tensor.matmul(
                    gt[:],
                    Wgs[:, dc, :],
                    xts[:, dc, :],
                    start=(dc == 0),
                    stop=(dc == DC - 1),
                )
            return gt

        def gate_exp(blk, gt):
            # exp rows at partitions 0-7; rows 32-39 get a copy via the DRAM
            # bounce so the second bias-seed matmul can use row group 1
            expv = epool.tile([32 + E, T], BF, name=f"exp{blk}", tag="exp")
            nc.scalar.activation(expv[0:E, :], gt[:], AF.Exp, bias=bgs[:, 0:1])
            # broadcast exp rows across partitions for the hidden scaling
            # (partition-stride-0 DMA only legal from DRAM -> bounce there)
            expd = dpool.tile([E, T], BF, name=f"expd{blk}", tag="expd")
            nc.gpsimd.dma_start(expd[:], expv[0:E, :])
            nc.gpsimd.dma_start(expv[32 : 32 + E, :], expd[:])
            bc = bcpool.tile([P, E, T], BF, name=f"bc{blk}", tag="bc")
            for e in range(E):
                nc.gpsimd.dma_start(bc[:, e, :], expd[e : e + 1, :].to_broadcast((P, T)))
            return expv, bc

        def gate_denom(blk, expv):
            # softmax denominator, landed in token-partition layout via a
            # K=8 ones matmul; both 128-token halves into one PSUM tile
            s = ps_s.tile([P, TS], FP, name=f"s{blk}", tag="s")
            for ts in range(TS):
                nc.tensor.matmul(
                    s[:, ts : ts + 1],
                    expv[0:E, ts * P : (ts + 1) * P],
                    ones8[:],
                    start=True,
                    stop=True,
                )
            rcp = rpool.tile([P, TS], FP, name=f"rcp{blk}", tag="rcp")
            nc.vector.reciprocal(rcp[:], s[:])
            return rcp

        def l1_stage(blk, xts, bc):
            sh = shpool.tile([P, NEJ, T], BF, name=f"sh{blk}", tag="sh")
            for ejc in range(NEJ):
                ht = ps_h.tile([P, T], FP, name=f"ht{blk}_{ejc}", tag="ht")
                for dc in range(DC):
                    nc.tensor.matmul(
                        ht[:],
                        W1s[:, dc, ejc * P : (ejc + 1) * P],
                        xts[:, dc, :],
                        start=(dc == 0),
                        stop=(dc == DC - 1),
                    )
                g = gpool.tile([P, T], BF, name=f"g{blk}_{ejc}", tag="g")
                nc.scalar.activation(g[:], ht[:], AF.Gelu, bias=b1s[:, ejc : ejc + 1])
                nc.vector.tensor_tensor(
                    sh[:, ejc, :], g[:], bc[:, ejc // JC_PER_E, :], mybir.AluOpType.mult
                )
            return sh

        # outputs never ride Pool: the software-DGE teardown DRAIN (~8.7us)
        # runs after Pool's last DMA, so a late Pool DMA lands it in the
        # measured tail
        OUT_ENG = [nc.sync, nc.scalar]

        def l2_stage(blk, sh, expv, rcp, last=False):
            t0 = blk * T
            for ts in range(TS):
                tsl = slice(ts * P, (ts + 1) * P)
                # the two K=8 bias seeds run concurrently in row groups 0/1
                ops = []
                for half in range(2):
                    o0 = half * OH
                    op = ps_o.tile(
                        [P, OH], FP, name=f"ops{blk}_{ts}_{half}", tag="ops"
                    )
                    r = 32 * half
                    nc.tensor.matmul(
                        op[:],
                        expv[r : r + E, tsl],
                        b2s[r : r + E, o0 : o0 + OH],
                        start=True,
                        stop=False,
                        tile_position=(r, 0),
                    )
                    ops.append(op)
                for half in range(2):
                    o0 = half * OH
                    for ejc in range(NEJ):
                        nc.tensor.matmul(
                            ops[half][:],
                            sh[:, ejc, tsl],
                            W2s[:, ejc, o0 : o0 + OH],
                            start=False,
                            stop=(ejc == NEJ - 1),
                        )
                for half in range(2):
                    o0 = half * OH
                    outsb = opool.tile(
                        [P, OH], FP, name=f"o{blk}_{ts}_{half}", tag="o"
                    )
                    # per-token 1/sum_e exp scale; DVE so ACT stays on gelu
                    nc.vector.tensor_scalar_mul(
                        outsb[:], ops[half][:], rcp[:, ts : ts + 1]
                    )
                    OUT_ENG[(ts + half) % 2].dma_start(
                        out[t0 + ts * P : t0 + (ts + 1) * P, o0 : o0 + OH], outsb[:]
                    )

        with tc.high_priority():
            gt0 = gate_logits(0, xtss[0])
            expv0, bc0 = gate_exp(0, gt0)
            # HAM warm-up bridge: the PE sits out ~2us here waiting for W1
            # chunk 0 (HBM-bound), which would keep the clock gate at
            # 1.2GHz deep into L1. Burn the wait on throwaway matmuls that
            # depend only on already-resident data so the activity monitor
            # sees a busy PE and releases full clock by the time real L1
            # work starts.
            dummy = ps_g.tile([E, T], FP, name="dummy", tag="gt")
            for k in range(14):
                nc.tensor.matmul(
                    dummy[:],
                    Wgs[:, k % DC, :],
                    xtss[0][:, (k + 1) % DC, :],
                    start=True,
                    stop=True,
                )
            rcp0 = gate_denom(0, expv0)
        states = {0: (expv0, rcp0, bc0)}
        for c in range(3, NCH):
            sl = slice(c * (EJ // NCH), (c + 1) * (EJ // NCH))
            nc.sync.dma_start(W1s[:, 0:4, sl], W1v[:, 0:4, sl])
            nc.scalar.dma_start(W1s[:, 4:6, sl], W1v[:, 4:6, sl])
            nc.gpsimd.dma_start(W1s[:, 6:8, sl], W1v[:, 6:8, sl])
        nc.sync.dma_start(b2s[0:E, :], b2[:])
        nc.sync.dma_start(b2s[32 : 32 + E, :], b2[:])
        xtss[1] = load_xts(1, split3=True)
        # W2 in single-ejc-chunk slices, consumption-ordered, weighted
        # toward the SP queue
        W2_ENG = [nc.sync, nc.gpsimd, nc.sync, nc.scalar]
        for sg in range(NEJ):
            W2_ENG[sg % 4].dma_start(W2s[:, sg : sg + 1, :], W2v[:, sg : sg + 1, :])

        sh_prev = None
        for blk in range(NB):
            sh = l1_stage(blk, xtss[blk], states[blk][2])
            if blk + 2 < NB:
                xtss[blk + 2] = load_xts(blk + 2)
            gt = gate_logits(blk + 1, xtss[blk + 1]) if blk + 1 < NB else None
            if blk >= 1:
                l2_stage(blk - 1, sh_prev, states[blk - 1][0], states[blk - 1][1])
            if blk + 1 < NB:
                expv, bc = gate_exp(blk + 1, gt)
                rcp = gate_denom(blk + 1, expv)
                states[blk + 1] = (expv, rcp, bc)
            sh_prev = sh
        l2_stage(NB - 1, sh_prev, states[NB - 1][0], states[NB - 1][1])
    return nc


_CACHE = {}


def kernel(**inputs):
    x = np.asarray(inputs["x"], dtype=np.float32)
    W1 = np.asarray(inputs["W1"], dtype=np.float32)
    b1 = np.asarray(inputs["b1"], dtype=np.float32)
    W2 = np.asarray(inputs["W2"], dtype=np.float32)
    b2 = np.asarray(inputs["b2"], dtype=np.float32)
    Wg = np.asarray(inputs["Wg"], dtype=np.float32)
    bg = np.asarray(inputs["bg"], dtype=np.float32)

    W1p = np.ascontiguousarray(W1.transpose(1, 0, 2).reshape(D, EJ)).astype(NPBF)
    W2p = np.ascontiguousarray(W2.reshape(EJ, O)).astype(NPBF)
    b1h = np.ascontiguousarray(b1.reshape(EJ).reshape(NEJ, P).T)
    bgh = np.ascontiguousarray(bg.reshape(E, 1))
    Wg16 = Wg.astype(NPBF)
    b216 = b2.astype(NPBF)

    if "nc" not in _CACHE:
        _CACHE["nc"] = _build_nc()
    nc = _CACHE["nc"]

    in_maps = []
    for c in range(NCORES):
        xs = x[c * NTOK : (c + 1) * NTOK]
        in_maps.append(
            {
                "xT": np.ascontiguousarray(xs.T).astype(NPBF),
                "W1p": W1p,
                "Wg": Wg16,
                "W2p": W2p,
                "b1h": b1h,
                "bgh": bgh,
                "b2": b216,
            }
        )

    res = run_bass_kernel_spmd(nc, in_maps, list(range(NCORES)))
    kernel.last = res
    return np.concatenate([res.results[c]["out"] for c in range(NCORES)], axis=0)
